# revision 24
# baseline (speedup 1.0000x reference)
"""Trainium2 Bass kernel for a fused MultiHead-GAT layer (8-core SPMD).

v2 strategy (edges sharded by sorted dst; tgt nodes data-parallel):
  host:  sort edges by dst, shard dst ranges across 8 cores, pad each
         128-tgt block's edge list to CPB 128-edge chunks, pre-transpose
         edge_embed chunks, pre-build one-hot chunks, fold attn_a into
         M1/V, precompute per-edge s1 = (src_h @ M1)[edge_src] so the
         score path never touches the gathered z rows. All device arrays
         are pre-shaped so every DMA is contiguous per partition.
  device (per core):
    phase0: z rows (natural h-major layout) -> zc_bounce -> AllGather to
    a [N_SRC, 512] bf16 shared table.
    score path (independent of z table, overlaps the AllGather):
    s2 = edge_embed^T @ V per chunk, e = leaky(s1+s2), eexp = exp(e).
    per tgt block: pipelined dma_gather of z rows (prepare_only +
    trigger_dma so desc-gen overlaps DMA flight; 512-elem rows), scale
    zg by eexp in place, one-hot matmul accumulates h and denominators,
    divide, elu (= relu(x) + min(exp(x),1) - 1) + residual, PE-transpose
    to feature-major, FFN (bf16), transpose back, LayerNorm via
    E[x^2]-E[x]^2 with Scalar-engine Square+accum, f32 output.
"""
import sys

sys.path.insert(0, "/opt/trn_rl_repo")

from contextlib import ExitStack
from types import SimpleNamespace

import numpy as np
import ml_dtypes

import concourse.bass as bass
import concourse.bacc as bacc
import concourse.tile as tile
from concourse import mybir

BF16 = mybir.dt.bfloat16
F32 = mybir.dt.float32
I16 = mybir.dt.int16
NP_BF16 = ml_dtypes.bfloat16

LN_EPS = 1e-5
LEAK = 0.01


def full_cfg():
    return SimpleNamespace(
        ncores=8,
        n_src=10000, n_tgt=10000, e=160000,
        in_dim=512, d=512, h=8, o=64, ed=128, fh=2048,
        tgt_per=1250, tgt_pad=1280, nblk=10,
    )


def host_prep(cfg, src_h, tgt_h, edge_embed, edge_src, edge_dst,
              W_fc, W_feat, attn_a, w1, b1, w2, b2, ln_g, ln_b):
    C = cfg
    H, O, D = C.h, C.o, C.d

    perm = np.argsort(edge_dst, kind="stable")
    es = np.asarray(edge_src)[perm].astype(np.int64)
    ed = np.asarray(edge_dst)[perm].astype(np.int64)
    ee = np.asarray(edge_embed)[perm]

    # feature permutation q = o*8+h  <->  f = h*64+o (z table in q-order so the
    # eexp broadcast multiply has a contiguous inner dim on the DVE)
    q = np.arange(D)
    f_of_q = (q % H) * O + (q // H)
    Wfc_p = np.asarray(W_fc)[:, f_of_q]

    a_src = np.asarray(attn_a)[0, :, :O]       # [H, O]
    a_feat = np.asarray(attn_a)[0, :, 2 * O:]  # [H, O]
    Ablk = np.zeros((D, H), np.float32)
    for h in range(H):
        Ablk[h * O:(h + 1) * O, h] = a_src[h]
    M1 = (np.asarray(W_fc, np.float64) @ Ablk.astype(np.float64)).astype(np.float32)
    V = np.zeros((C.ed, H), np.float32)
    for h in range(H):
        V[:, h] = np.asarray(W_feat)[:, h * O:(h + 1) * O] @ a_feat[h]
    # per-edge s1 scores, computed in f64 on host (linear projection + gather)
    s1_nodes = (np.asarray(src_h, np.float64) @ M1.astype(np.float64)).astype(np.float32)
    s1_edge = s1_nodes[es]                     # [E, H]

    # global 128-tgt blocks, balanced across cores per slot (slot s of every
    # core gets blocks of similar edge count; per-slot chunk count = max/128)
    block_bounds = []
    for gb in range(C.ncores * C.nblk):
        lo = gb * 128
        hi = min(lo + 128, C.n_tgt)
        block_bounds.append((lo, hi))
    cnts = np.array([np.searchsorted(ed, hi) - np.searchsorted(ed, lo)
                     for (lo, hi) in block_bounds])
    order = np.argsort(-cnts, kind="stable")
    cpbs = []
    assign = [[None] * C.nblk for _ in range(C.ncores)]
    for s in range(C.nblk):
        group = order[s * C.ncores:(s + 1) * C.ncores]
        cpbs.append(max(1, int((cnts[group].max() + 127) // 128)))
        for c in range(C.ncores):
            assign[c][s] = int(group[c])
    C.cpbs = tuple(cpbs)
    coff = np.concatenate([[0], np.cumsum(cpbs)]).astype(int)
    TOTC = int(coff[-1])
    C.totc = TOTC

    KT = C.in_dim // 128
    MT1 = C.fh // 128
    FT = D // 128

    cores = []
    for c in range(C.ncores):
        idxw = np.zeros((128, TOTC * 8), np.int16)
        eeT = np.zeros((128, TOTC, 128), NP_BF16)
        oh = np.zeros((128, TOTC, 128), NP_BF16)
        s1b = np.zeros((128, TOTC, 8), NP_BF16)
        th = np.zeros((C.nblk * 128, D), np.float32)
        bounds = []
        for b in range(C.nblk):
            cpb = cpbs[b]
            o0 = int(coff[b])
            lo, hi = block_bounds[assign[c][b]]
            bounds.append((lo, hi))
            th[b * 128:b * 128 + hi - lo] = np.asarray(tgt_h)[lo:hi] - 1.0
            s, t = np.searchsorted(ed, lo), np.searchsorted(ed, hi)
            n = t - s
            src_b = np.zeros(cpb * 128, np.int64)
            src_b[:n] = es[s:t]
            lt = np.full(cpb * 128, -1, np.int64)
            lt[:n] = ed[s:t] - lo
            # gather index wrap: logical i -> partition i%16, col i//16, x8 replicated
            base = src_b.astype(np.int16).reshape(-1, 16).T  # [16, cpb*8]
            for k in range(8):
                idxw[k * 16:(k + 1) * 16, o0 * 8:(o0 + cpb) * 8] = base
            eb = np.zeros((cpb * 128, C.ed), NP_BF16)
            eb[:n] = ee[s:t].astype(NP_BF16)
            eeT[:, o0:o0 + cpb, :] = eb.reshape(cpb, 128, C.ed).transpose(2, 0, 1)
            ohb = np.zeros((cpb * 128, 128), NP_BF16)
            valid = lt >= 0
            ohb[np.nonzero(valid)[0], lt[valid]] = 1.0
            oh[:, o0:o0 + cpb, :] = ohb.reshape(cpb, 128, 128).transpose(1, 0, 2)
            sb = np.zeros((cpb * 128, 8), NP_BF16)
            sb[:n] = s1_edge[s:t].astype(NP_BF16)
            s1b[:, o0:o0 + cpb, :] = sb.reshape(cpb, 128, 8).transpose(1, 0, 2)

        # src_hT pre-shaped [128, KT, NPAD] (src shard unchanged by balancing)
        sh = np.zeros((128, KT, C.tgt_pad), np.float32)
        lo2 = c * C.tgt_per
        hi2 = min((c + 1) * C.tgt_per, C.n_src)
        nrows = hi2 - lo2
        blk = np.asarray(src_h)[lo2:hi2]                 # [nrows, 512]
        sh[:, :, :nrows] = blk.T.reshape(KT, 128, nrows).transpose(1, 0, 2)

        cores.append({
            "idxw": idxw, "eeT": eeT, "oh": oh, "s1b": s1b,
            "tgt_hm1": th.astype(NP_BF16),
            "src_hT": sh.astype(NP_BF16),
            "bounds": bounds,
        })

    def pshape(w, kt):
        # [kt*128, M] -> [128, kt, M]
        w = np.asarray(w)
        return np.ascontiguousarray(
            w.reshape(kt, 128, w.shape[1]).transpose(1, 0, 2))

    shared = {
        "wfc": pshape(Wfc_p, KT).astype(NP_BF16),           # [128, KT, 512]
        "v": np.asarray(V).astype(NP_BF16),                 # [128, 8]
        "w1": pshape(np.asarray(w1), KT).reshape(
            128, KT, MT1, 128).astype(NP_BF16),
        "w2": pshape(np.asarray(w2), MT1).reshape(
            128, MT1, FT, 128).astype(NP_BF16),
        "b1c": np.ascontiguousarray(
            np.asarray(b1, np.float32).reshape(MT1, 128).T),  # [128, MT1]
        "b2c": np.ascontiguousarray(
            np.asarray(b2, np.float32).reshape(FT, 128).T),   # [128, FT]
        "g_rep": np.tile(np.asarray(ln_g, NP_BF16).reshape(1, D), (128, 1)),
        "b_rep": np.tile(np.asarray(ln_b, np.float32).reshape(1, D), (128, 1)),
        "identb": np.eye(128, dtype=NP_BF16),
    }
    return cores, shared


def build_program(C):
    nc = bacc.Bacc("TRN2", target_bir_lowering=False, debug=False,
                   num_devices=C.ncores)
    H, O, D, NBLK = C.h, C.o, C.d, C.nblk
    CPBS, TOTC = C.cpbs, C.totc
    MAXC = max(CPBS)
    COFF = [0]
    for x in CPBS:
        COFF.append(COFF[-1] + x)
    NPAD = C.tgt_pad
    KT = C.in_dim // 128   # 4
    FT = D // 128          # 4
    MT1 = C.fh // 128      # 16

    def din(name, shape, dt):
        return nc.dram_tensor(name, shape, dt, kind="ExternalInput").ap()

    idxw = din("idxw", [128, TOTC * 8], I16)
    eeT = din("eeT", [128, TOTC, 128], BF16)
    oh = din("oh", [128, TOTC, 128], BF16)
    s1b = din("s1b", [128, TOTC, 8], BF16)
    tgt_hm1 = din("tgt_hm1", [NPAD, D], BF16)
    src_hT = din("src_hT", [128, KT, NPAD], BF16)
    wfc = din("wfc", [128, KT, D], BF16)
    vmat = din("v", [C.ed, H], BF16)
    w1 = din("w1", [128, KT, MT1, 128], BF16)
    w2 = din("w2", [128, MT1, FT, 128], BF16)
    b1c = din("b1c", [128, MT1], F32)
    b2c = din("b2c", [128, FT], F32)
    g_rep = din("g_rep", [128, D], BF16)
    b_rep = din("b_rep", [128, D], F32)
    identb = din("identb", [128, 128], BF16)

    out_shard = nc.dram_tensor("out_shard", [NPAD, D], F32,
                               kind="ExternalOutput").ap()

    zc_bounce = nc.dram_tensor("zc_bounce", [C.tgt_per, D], BF16).ap()
    zc_space = "Shared" if C.ncores > 4 else None
    zc_table = nc.dram_tensor("zc_table", [C.n_src, D], BF16,
                              addr_space=zc_space).ap()

    with tile.TileContext(nc) as tc, ExitStack() as top:
        const = top.enter_context(tc.tile_pool(name="const", bufs=1))

        wfc_sb = const.tile([128, KT, D], BF16)
        nc.sync.dma_start(wfc_sb[:], wfc[:, :, :])

        # ---------------- phase 0: z rows -> zc_bounce -> AllGather
        with ExitStack() as p0:
            ps0 = p0.enter_context(tc.tile_pool(name="ps0", bufs=2, space="PSUM"))
            zr_pool = p0.enter_context(tc.tile_pool(name="zrow", bufs=2))
            shp = p0.enter_context(tc.tile_pool(name="shp", bufs=1))
            sh_sb = shp.tile([128, KT, NPAD], BF16)
            nc.sync.dma_start(sh_sb[:], src_hT[:, :, :])
            for nb in range(NBLK):
                rows = min(128, C.tgt_per - nb * 128)
                if rows <= 0:
                    break
                z_ps = ps0.tile([128, D], F32, tag="zps")
                for kt in range(KT):
                    nc.tensor.matmul(z_ps[:], sh_sb[:, kt, nb * 128:(nb + 1) * 128],
                                     wfc_sb[:, kt, :], start=(kt == 0),
                                     stop=(kt == KT - 1))
                zrow = zr_pool.tile([128, D], BF16, tag="zrow")
                nc.scalar.activation(zrow[:], z_ps[:],
                                     mybir.ActivationFunctionType.Copy)
                nc.sync.dma_start(zc_bounce[nb * 128:nb * 128 + rows, :],
                                  zrow[0:rows, :])

        nc.gpsimd.collective_compute(
            "AllGather", mybir.AluOpType.bypass,
            replica_groups=[list(range(C.ncores))],
            ins=[zc_bounce[:, :]], outs=[zc_table[:, :]],
        )

        # small consts (needed right at CC end; negligible traffic)
        v_sb = const.tile([128, H], BF16)
        nc.sync.dma_start(v_sb[:], vmat[:, :])
        idx_sb = const.tile([128, TOTC * 8], I16)
        nc.sync.dma_start(idx_sb[:], idxw[:, :])
        b1_sb = const.tile([128, MT1], F32)
        nc.sync.dma_start(b1_sb[:], b1c[:, :])
        b2_sb = const.tile([128, FT], F32)
        nc.sync.dma_start(b2_sb[:], b2c[:, :])
        grep_sb = const.tile([128, D], BF16)
        nc.sync.dma_start(grep_sb[:], g_rep[:, :])
        brep_sb = const.tile([128, D], F32)
        nc.sync.dma_start(brep_sb[:], b_rep[:, :])
        idb_sb = const.tile([128, 128], BF16)
        nc.sync.dma_start(idb_sb[:], identb[:, :])
        eps_sb = const.tile([128, 1], F32)
        nc.vector.memset(eps_sb[:], LN_EPS)
        ones_sb = const.tile([128, D], BF16)
        nc.vector.memset(ones_sb[:], 1.0)

        # gate: RAW on the collective; the in-order sync queue then holds all
        # later dma_starts until the CC is done, keeping the CC exchange free
        # of heavy DMA contention
        ccgate = const.tile([128, D], BF16)
        nc.sync.dma_start(ccgate[:], zc_table[0:128, :])

        w1_sb = const.tile([128, KT, MT1, 128], BF16)
        nc.sync.dma_start(w1_sb[:], w1[:, :, :, :])
        w2_sb = const.tile([128, MT1, FT, 128], BF16)
        nc.sync.dma_start(w2_sb[:], w2[:, :, :, :])

        with ExitStack() as pb:
            keep = pb.enter_context(tc.tile_pool(name="keep", bufs=NBLK))

            # -------- score path: independent of the z table / AllGather,
            # emitted before the collective so its DMA doesn't contend
            eexps = []
            with ExitStack() as sp:
                ps_s2 = sp.enter_context(
                    tc.tile_pool(name="ps_s2", bufs=1, space="PSUM"))
                eep = sp.enter_context(tc.tile_pool(name="eep", bufs=2))
                s1pp = sp.enter_context(tc.tile_pool(name="s1p", bufs=2))
                e1p = sp.enter_context(tc.tile_pool(name="e1p", bufs=2))
                for nb in range(NBLK):
                    CPB = CPBS[nb]
                    o0 = COFF[nb]
                    ee_t = eep.tile([128, MAXC, 128], BF16, tag="ee")
                    nc.sync.dma_start(ee_t[:, 0:CPB, :], eeT[:, o0:o0 + CPB, :])
                    s1_t = s1pp.tile([128, MAXC, 8], BF16, tag="s1")
                    nc.sync.dma_start(s1_t[:, 0:CPB, :], s1b[:, o0:o0 + CPB, :])
                    s2_ps = ps_s2.tile([128, MAXC * H], F32, tag="s2")
                    for j in range(CPB):
                        nc.tensor.matmul(s2_ps[:, j * H:(j + 1) * H], ee_t[:, j, :],
                                         v_sb[:, :], start=True, stop=True)
                    e1 = e1p.tile([128, MAXC, H], F32, tag="e1")
                    nc.vector.tensor_tensor(
                        e1[:, 0:CPB, :],
                        s2_ps[:, 0:CPB * H].rearrange("p (c h) -> p c h", h=H),
                        s1_t[:, 0:CPB, :], mybir.AluOpType.add)
                    lk = e1p.tile([128, MAXC, H], F32, tag="lk")
                    nc.vector.tensor_scalar_mul(lk[:, 0:CPB, :], e1[:, 0:CPB, :],
                                                LEAK)
                    e2 = e1p.tile([128, MAXC, H], F32, tag="e2")
                    nc.vector.tensor_tensor(e2[:, 0:CPB, :], e1[:, 0:CPB, :],
                                            lk[:, 0:CPB, :], mybir.AluOpType.max)
                    eexp = keep.tile([128, MAXC, H], BF16, tag="eexp")
                    nc.scalar.activation(eexp[:, 0:CPB, :], e2[:, 0:CPB, :],
                                         mybir.ActivationFunctionType.Exp)
                    eexps.append(eexp)

            ps_hag = pb.enter_context(tc.tile_pool(name="ps_hag", bufs=1, space="PSUM"))
            ps_den = pb.enter_context(tc.tile_pool(name="ps_den", bufs=1, space="PSUM"))
            ps_tp = pb.enter_context(tc.tile_pool(name="ps_tp", bufs=1, space="PSUM"))
            ps_a1 = pb.enter_context(tc.tile_pool(name="ps_a1", bufs=3, space="PSUM"))
            ps_o2 = pb.enter_context(tc.tile_pool(name="ps_o2", bufs=2, space="PSUM"))
            ohp = pb.enter_context(tc.tile_pool(name="ohp", bufs=3))
            tgtp = pb.enter_context(tc.tile_pool(name="tgtp", bufs=2))
            zgp = pb.enter_context(tc.tile_pool(name="zg", bufs=4))
            hpool = pb.enter_context(tc.tile_pool(name="hb", bufs=2))
            hbtp = pb.enter_context(tc.tile_pool(name="hbt", bufs=1))
            fpool = pb.enter_context(tc.tile_pool(name="ffn", bufs=1))
            r1p = pb.enter_context(tc.tile_pool(name="r1", bufs=1))
            tmpp = pb.enter_context(tc.tile_pool(name="tmp", bufs=2))
            lnp = pb.enter_context(tc.tile_pool(name="ln", bufs=2))
            stp = pb.enter_context(tc.tile_pool(name="stat", bufs=2))

            hbT = hbtp.tile([128, FT, NPAD], BF16)
            r2 = fpool.tile([128, FT, NPAD], BF16)

            # -------- main block loop
            GMAX = 8  # 1024 idxs per gather; 640/768 proven on HW, 1152+ crashes
            ffn_plan = {1: [(0, 256)], 3: [(256, 256)], 5: [(512, 256)],
                        7: [(768, 256)], 9: [(1024, 256)]}

            def emit_ffn_chunk(cs, cw):
                r1 = r1p.tile([128, MT1, 512], BF16, tag="r1")
                for mt in range(MT1):
                    a1 = ps_a1.tile([128, cw], F32, tag="a1")
                    for kt in range(KT):
                        nc.tensor.matmul(a1[:], w1_sb[:, kt, mt, :],
                                         hbT[:, kt, cs:cs + cw],
                                         start=(kt == 0), stop=(kt == KT - 1))
                    nc.scalar.activation(r1[:, mt, 0:cw], a1[:],
                                         mybir.ActivationFunctionType.Relu,
                                         bias=b1_sb[:, mt:mt + 1])
                for ft in range(FT):
                    o2 = ps_o2.tile([128, cw], F32, tag="o2")
                    for kt2 in range(MT1):
                        nc.tensor.matmul(o2[:], w2_sb[:, kt2, ft, :],
                                         r1[:, kt2, 0:cw],
                                         start=(kt2 == 0), stop=(kt2 == MT1 - 1))
                    t1 = tmpp.tile([128, cw], BF16, tag="t1")
                    nc.scalar.activation(t1[:], o2[:],
                                         mybir.ActivationFunctionType.Identity,
                                         bias=b2_sb[:, ft:ft + 1])
                    nc.vector.tensor_tensor(r2[:, ft, cs:cs + cw], t1[:],
                                            hbT[:, ft, cs:cs + cw],
                                            mybir.AluOpType.add)

            def emit_ln(nb):
                tp2 = ps_tp.tile([128, D], BF16, tag="tp")
                for ft in range(FT):
                    nc.tensor.transpose(tp2[:, ft * 128:(ft + 1) * 128],
                                        r2[:, ft, nb * 128:(nb + 1) * 128],
                                        idb_sb[:])
                r2n = lnp.tile([128, D], BF16, tag="r2n")
                nc.vector.tensor_copy(r2n[:], tp2[:])
                ssum = stp.tile([128, 1], F32, tag="ssum")
                nc.vector.tensor_reduce(ssum[:], r2n[:], mybir.AxisListType.X,
                                        mybir.AluOpType.add)
                sqs = lnp.tile([128, D], BF16, tag="sqs")
                ssq = stp.tile([128, 1], F32, tag="ssq")
                nc.scalar.activation(sqs[:], r2n[:],
                                     mybir.ActivationFunctionType.Square,
                                     accum_out=ssq[:])
                mu_n = stp.tile([128, 1], F32, tag="mu_n")
                nc.vector.tensor_scalar_mul(mu_n[:], ssum[:], -1.0 / D)
                musq = stp.tile([128, 1], F32, tag="musq")
                nc.vector.tensor_tensor(musq[:], mu_n[:], mu_n[:],
                                        mybir.AluOpType.mult)
                var = stp.tile([128, 1], F32, tag="var")
                nc.vector.tensor_scalar(var[:], ssq[:], 1.0 / D, None,
                                        mybir.AluOpType.mult)
                var2 = stp.tile([128, 1], F32, tag="var2")
                nc.vector.tensor_tensor(var2[:], var[:], musq[:],
                                        mybir.AluOpType.subtract)
                std = stp.tile([128, 1], F32, tag="std")
                nc.scalar.activation(std[:], var2[:],
                                     mybir.ActivationFunctionType.Sqrt,
                                     bias=eps_sb[:, :])
                rstd = stp.tile([128, 1], F32, tag="rstd")
                nc.vector.reciprocal(rstd[:], std[:])
                nsh = stp.tile([128, 1], F32, tag="nsh")
                nc.vector.tensor_tensor(nsh[:], mu_n[:], rstd[:],
                                        mybir.AluOpType.mult)
                y = lnp.tile([128, D], BF16, tag="y")
                nc.vector.tensor_scalar(y[:], r2n[:], rstd[:], nsh[:],
                                        mybir.AluOpType.mult,
                                        mybir.AluOpType.add)
                xg = lnp.tile([128, D], BF16, tag="xg")
                nc.vector.tensor_tensor(xg[:], y[:], grep_sb[:],
                                        mybir.AluOpType.mult)
                orow = lnp.tile([128, D], F32, tag="orow")
                nc.vector.tensor_tensor(orow[:], xg[:], brep_sb[:],
                                        mybir.AluOpType.add)
                nc.sync.dma_start(out_shard[nb * 128:(nb + 1) * 128, :], orow[:])

            for nb in range(NBLK):
                CPB = CPBS[nb]
                o0 = COFF[nb]
                zg = zgp.tile([128, MAXC, D], BF16, tag="zg")
                for g0 in range(0, CPB, GMAX):
                    gn = min(GMAX, CPB - g0)
                    nc.gpsimd.dma_gather(
                        out_ap=zg[:, g0:g0 + gn, :], in_ap=zc_table[:, :],
                        idxs_ap=idx_sb[:, (o0 + g0) * 8:(o0 + g0 + gn) * 8],
                        num_idxs=gn * 128, num_idxs_reg=gn * 128, elem_size=D)
                oh_t = ohp.tile([128, MAXC, 128], BF16, tag="oh")
                nc.sync.dma_start(oh_t[:, 0:CPB, :], oh[:, o0:o0 + CPB, :])
                tgtb = tgtp.tile([128, D], BF16, tag="tgtb")
                nc.sync.dma_start(tgtb[:], tgt_hm1[nb * 128:(nb + 1) * 128, :])

                eexp = eexps[nb]
                # zg *= eexp (q-order: inner dim h contiguous on both), in place
                nc.vector.tensor_tensor(
                    zg[:, 0:CPB, :].rearrange("p c (o h) -> p c o h", h=H),
                    zg[:, 0:CPB, :].rearrange("p c (o h) -> p c o h", h=H),
                    eexp[:, 0:CPB, :].rearrange("p c (h x) -> p c x h", x=1)
                        .broadcast_to([128, CPB, O, H]),
                    mybir.AluOpType.mult)

                hag = ps_hag.tile([128, D], F32, tag="hag")
                den = ps_den.tile([128, H], F32, tag="den")
                for j in range(CPB):
                    nc.tensor.matmul(hag[:], oh_t[:, j, :], zg[:, j, :],
                                     start=(j == 0), stop=(j == CPB - 1),
                                     skip_group_check=True)
                    nc.tensor.matmul(den[:], oh_t[:, j, :], eexp[:, j, :],
                                     start=(j == 0), stop=(j == CPB - 1),
                                     skip_group_check=True)

                den2 = stp.tile([128, H], F32, tag="den2")
                nc.vector.tensor_scalar_max(den2[:], den[:], 1e-30)
                rec = stp.tile([128, H], F32, tag="rec")
                nc.vector.reciprocal(rec[:], den2[:])

                hbp = hpool.tile([128, D], BF16, tag="hbp")
                nc.vector.tensor_tensor(
                    hbp[:, :].rearrange("p (h o) -> p h o", h=H),
                    hag[:, :].rearrange("p (o h) -> p h o", h=H),
                    rec[:, :].rearrange("p (h x) -> p h x", x=1)
                        .broadcast_to([128, H, O]),
                    mybir.AluOpType.mult)
                # elu(x) + tgt = relu(x) + min(exp(x),1) - 1 + tgt
                ex = hpool.tile([128, D], BF16, tag="ex")
                nc.scalar.activation(ex[:], hbp[:],
                                     mybir.ActivationFunctionType.Exp)
                u = hpool.tile([128, D], BF16, tag="u")
                nc.vector.tensor_tensor(u[:], ex[:], ones_sb[:],
                                        mybir.AluOpType.min)
                v = hpool.tile([128, D], BF16, tag="v")
                nc.scalar.activation(v[:], hbp[:],
                                     mybir.ActivationFunctionType.Relu)
                w = hpool.tile([128, D], BF16, tag="w")
                nc.vector.tensor_tensor(w[:], u[:], tgtb[:],
                                        mybir.AluOpType.add)
                hb3 = hpool.tile([128, D], BF16, tag="hb3")
                nc.vector.tensor_tensor(hb3[:], w[:], v[:],
                                        mybir.AluOpType.add)

                tp = ps_tp.tile([128, D], BF16, tag="tp")
                for ft in range(FT):
                    nc.tensor.transpose(tp[:, ft * 128:(ft + 1) * 128],
                                        hb3[:, ft * 128:(ft + 1) * 128],
                                        idb_sb[:])
                nc.vector.tensor_copy(
                    hbT[:, :, nb * 128:(nb + 1) * 128],
                    tp[:].rearrange("p (f m) -> p f m", m=128))

                for cs, cw in ffn_plan.get(nb, []):
                    emit_ffn_chunk(cs, cw)
                    for nb_ln in range(cs // 128, (cs + cw) // 128):
                        emit_ln(nb_ln)

    nc.compile()
    return nc


_CACHE = {}


def _get_program(C):
    key = (C.ncores, C.n_src, C.n_tgt, C.e, C.cpbs)
    if key not in _CACHE:
        _CACHE[key] = build_program(C)
    return _CACHE[key]


def kernel(src_h, tgt_h, edge_embed, edge_src, edge_dst,
           W_fc, W_feat, attn_a, w1, b1, w2, b2, ln_g, ln_b):
    from concourse.bass_utils import run_bass_kernel_spmd

    C = full_cfg()
    cores, shared = host_prep(C, src_h, tgt_h, edge_embed, edge_src, edge_dst,
                              W_fc, W_feat, attn_a, w1, b1, w2, b2, ln_g, ln_b)
    nc = _get_program(C)
    in_maps = []
    for c in range(C.ncores):
        m = dict(shared)
        cc = cores[c]
        m.update(idxw=cc["idxw"], eeT=cc["eeT"], oh=cc["oh"], s1b=cc["s1b"],
                 tgt_hm1=cc["tgt_hm1"], src_hT=cc["src_hT"])
        in_maps.append(m)
    import os
    try:
        res = run_bass_kernel_spmd(nc, in_maps, list(range(C.ncores)))
    except Exception:
        if os.environ.get("BASS_TRACE"):
            os.environ["BASS_NEVER_TRACE"] = "1"
            res = run_bass_kernel_spmd(nc, in_maps, list(range(C.ncores)))
        else:
            raise
    global _last_results
    _last_results = res
    out = np.zeros((C.n_tgt, C.d), np.float32)
    for c in range(C.ncores):
        shard = res.results[c]["out_shard"]
        for s, (lo, hi) in enumerate(cores[c]["bounds"]):
            if hi > lo:
                out[lo:hi] = shard[s * 128:s * 128 + (hi - lo)]
    return np.ascontiguousarray(out, dtype=np.float32)


# revision 25
# speedup vs baseline: 1.0758x; 1.0758x over previous
"""Trainium2 Bass kernel for a fused MultiHead-GAT layer (8-core SPMD).

v2 strategy (edges sharded by sorted dst; tgt nodes data-parallel):
  host:  sort edges by dst, shard dst ranges across 8 cores, pad each
         128-tgt block's edge list to CPB 128-edge chunks, pre-transpose
         edge_embed chunks, pre-build one-hot chunks, fold attn_a into
         M1/V, precompute per-edge s1 = (src_h @ M1)[edge_src] so the
         score path never touches the gathered z rows. All device arrays
         are pre-shaped so every DMA is contiguous per partition.
  device (per core):
    phase0: z rows (natural h-major layout) -> zc_bounce -> AllGather to
    a [N_SRC, 512] bf16 shared table.
    score path (independent of z table, overlaps the AllGather):
    s2 = edge_embed^T @ V per chunk, e = leaky(s1+s2), eexp = exp(e).
    per tgt block: pipelined dma_gather of z rows (prepare_only +
    trigger_dma so desc-gen overlaps DMA flight; 512-elem rows), scale
    zg by eexp in place, one-hot matmul accumulates h and denominators,
    divide, elu (= relu(x) + min(exp(x),1) - 1) + residual, PE-transpose
    to feature-major, FFN (bf16), transpose back, LayerNorm via
    E[x^2]-E[x]^2 with Scalar-engine Square+accum, f32 output.
"""
import sys

sys.path.insert(0, "/opt/trn_rl_repo")

from contextlib import ExitStack
from types import SimpleNamespace

import numpy as np
import ml_dtypes

import concourse.bass as bass
import concourse.bacc as bacc
import concourse.tile as tile
from concourse import mybir

BF16 = mybir.dt.bfloat16
F32 = mybir.dt.float32
I16 = mybir.dt.int16
NP_BF16 = ml_dtypes.bfloat16

LN_EPS = 1e-5
LEAK = 0.01


def full_cfg():
    return SimpleNamespace(
        ncores=8,
        n_src=10000, n_tgt=10000, e=160000,
        in_dim=512, d=512, h=8, o=64, ed=128, fh=2048,
        tgt_per=1250, tgt_pad=1280, nblk=10,
    )


def host_prep(cfg, src_h, tgt_h, edge_embed, edge_src, edge_dst,
              W_fc, W_feat, attn_a, w1, b1, w2, b2, ln_g, ln_b):
    C = cfg
    H, O, D = C.h, C.o, C.d

    perm = np.argsort(edge_dst, kind="stable")
    es = np.asarray(edge_src)[perm].astype(np.int64)
    ed = np.asarray(edge_dst)[perm].astype(np.int64)
    ee = np.asarray(edge_embed)[perm]

    # feature permutation q = o*8+h  <->  f = h*64+o (z table in q-order so the
    # eexp broadcast multiply has a contiguous inner dim on the DVE)
    q = np.arange(D)
    f_of_q = (q % H) * O + (q // H)
    Wfc_p = np.asarray(W_fc)[:, f_of_q]

    a_src = np.asarray(attn_a)[0, :, :O]       # [H, O]
    a_feat = np.asarray(attn_a)[0, :, 2 * O:]  # [H, O]
    Ablk = np.zeros((D, H), np.float32)
    for h in range(H):
        Ablk[h * O:(h + 1) * O, h] = a_src[h]
    M1 = (np.asarray(W_fc, np.float64) @ Ablk.astype(np.float64)).astype(np.float32)
    V = np.zeros((C.ed, H), np.float32)
    for h in range(H):
        V[:, h] = np.asarray(W_feat)[:, h * O:(h + 1) * O] @ a_feat[h]
    # per-edge s1 scores, computed in f64 on host (linear projection + gather)
    s1_nodes = (np.asarray(src_h, np.float64) @ M1.astype(np.float64)).astype(np.float32)
    s1_edge = s1_nodes[es]                     # [E, H]

    # global 128-tgt blocks, balanced across cores per slot (slot s of every
    # core gets blocks of similar edge count; per-slot chunk count = max/128)
    block_bounds = []
    for gb in range(C.ncores * C.nblk):
        lo = gb * 128
        hi = min(lo + 128, C.n_tgt)
        block_bounds.append((lo, hi))
    cnts = np.array([np.searchsorted(ed, hi) - np.searchsorted(ed, lo)
                     for (lo, hi) in block_bounds])
    order = np.argsort(-cnts, kind="stable")
    cpbs = []
    assign = [[None] * C.nblk for _ in range(C.ncores)]
    for s in range(C.nblk):
        group = order[s * C.ncores:(s + 1) * C.ncores]
        cpbs.append(max(1, int((cnts[group].max() + 127) // 128)))
        for c in range(C.ncores):
            assign[c][s] = int(group[c])
    C.cpbs = tuple(cpbs)
    coff = np.concatenate([[0], np.cumsum(cpbs)]).astype(int)
    TOTC = int(coff[-1])
    C.totc = TOTC

    KT = C.in_dim // 128
    MT1 = C.fh // 128
    FT = D // 128

    cores = []
    for c in range(C.ncores):
        idxw = np.zeros((128, TOTC * 8), np.int16)
        eeT = np.zeros((128, TOTC, 128), NP_BF16)
        oh = np.zeros((128, TOTC, 128), NP_BF16)
        s1b = np.zeros((128, TOTC, 8), NP_BF16)
        th = np.zeros((C.nblk * 128, D), np.float32)
        bounds = []
        for b in range(C.nblk):
            cpb = cpbs[b]
            o0 = int(coff[b])
            lo, hi = block_bounds[assign[c][b]]
            bounds.append((lo, hi))
            th[b * 128:b * 128 + hi - lo] = np.asarray(tgt_h)[lo:hi] - 1.0
            s, t = np.searchsorted(ed, lo), np.searchsorted(ed, hi)
            n = t - s
            src_b = np.zeros(cpb * 128, np.int64)
            src_b[:n] = es[s:t]
            lt = np.full(cpb * 128, -1, np.int64)
            lt[:n] = ed[s:t] - lo
            # gather index wrap: logical i -> partition i%16, col i//16, x8 replicated
            base = src_b.astype(np.int16).reshape(-1, 16).T  # [16, cpb*8]
            for k in range(8):
                idxw[k * 16:(k + 1) * 16, o0 * 8:(o0 + cpb) * 8] = base
            eb = np.zeros((cpb * 128, C.ed), NP_BF16)
            eb[:n] = ee[s:t].astype(NP_BF16)
            eeT[:, o0:o0 + cpb, :] = eb.reshape(cpb, 128, C.ed).transpose(2, 0, 1)
            ohb = np.zeros((cpb * 128, 128), NP_BF16)
            valid = lt >= 0
            ohb[np.nonzero(valid)[0], lt[valid]] = 1.0
            oh[:, o0:o0 + cpb, :] = ohb.reshape(cpb, 128, 128).transpose(1, 0, 2)
            sb = np.zeros((cpb * 128, 8), NP_BF16)
            sb[:n] = s1_edge[s:t].astype(NP_BF16)
            s1b[:, o0:o0 + cpb, :] = sb.reshape(cpb, 128, 8).transpose(1, 0, 2)

        # src_hT pre-shaped [128, KT, NPAD] (src shard unchanged by balancing)
        sh = np.zeros((128, KT, C.tgt_pad), np.float32)
        lo2 = c * C.tgt_per
        hi2 = min((c + 1) * C.tgt_per, C.n_src)
        nrows = hi2 - lo2
        blk = np.asarray(src_h)[lo2:hi2]                 # [nrows, 512]
        sh[:, :, :nrows] = blk.T.reshape(KT, 128, nrows).transpose(1, 0, 2)

        cores.append({
            "idxw": idxw, "eeT": eeT, "oh": oh, "s1b": s1b,
            "tgt_hm1": th.astype(NP_BF16),
            "src_hT": sh.astype(NP_BF16),
            "bounds": bounds,
        })

    def pshape(w, kt):
        # [kt*128, M] -> [128, kt, M]
        w = np.asarray(w)
        return np.ascontiguousarray(
            w.reshape(kt, 128, w.shape[1]).transpose(1, 0, 2))

    shared = {
        "wfc": pshape(Wfc_p, KT).astype(NP_BF16),           # [128, KT, 512]
        "v": np.asarray(V).astype(NP_BF16),                 # [128, 8]
        "w1": pshape(np.asarray(w1), KT).reshape(
            128, KT, MT1, 128).astype(NP_BF16),
        "w2": pshape(np.asarray(w2), MT1).reshape(
            128, MT1, FT, 128).astype(NP_BF16),
        "b1c": np.ascontiguousarray(
            np.asarray(b1, np.float32).reshape(MT1, 128).T),  # [128, MT1]
        "b2c": np.ascontiguousarray(
            np.asarray(b2, np.float32).reshape(FT, 128).T),   # [128, FT]
        "g_rep": np.tile(np.asarray(ln_g, NP_BF16).reshape(1, D), (128, 1)),
        "b_rep": np.tile(np.asarray(ln_b, np.float32).reshape(1, D), (128, 1)),
        "identb": np.eye(128, dtype=NP_BF16),
    }
    return cores, shared


def build_program(C):
    nc = bacc.Bacc("TRN2", target_bir_lowering=False, debug=False,
                   num_devices=C.ncores)
    H, O, D, NBLK = C.h, C.o, C.d, C.nblk
    CPBS, TOTC = C.cpbs, C.totc
    MAXC = max(CPBS)
    COFF = [0]
    for x in CPBS:
        COFF.append(COFF[-1] + x)
    NPAD = C.tgt_pad
    KT = C.in_dim // 128   # 4
    FT = D // 128          # 4
    MT1 = C.fh // 128      # 16

    def din(name, shape, dt):
        return nc.dram_tensor(name, shape, dt, kind="ExternalInput").ap()

    idxw = din("idxw", [128, TOTC * 8], I16)
    eeT = din("eeT", [128, TOTC, 128], BF16)
    oh = din("oh", [128, TOTC, 128], BF16)
    s1b = din("s1b", [128, TOTC, 8], BF16)
    tgt_hm1 = din("tgt_hm1", [NPAD, D], BF16)
    src_hT = din("src_hT", [128, KT, NPAD], BF16)
    wfc = din("wfc", [128, KT, D], BF16)
    vmat = din("v", [C.ed, H], BF16)
    w1 = din("w1", [128, KT, MT1, 128], BF16)
    w2 = din("w2", [128, MT1, FT, 128], BF16)
    b1c = din("b1c", [128, MT1], F32)
    b2c = din("b2c", [128, FT], F32)
    g_rep = din("g_rep", [128, D], BF16)
    b_rep = din("b_rep", [128, D], F32)
    identb = din("identb", [128, 128], BF16)

    out_shard = nc.dram_tensor("out_shard", [NPAD, D], F32,
                               kind="ExternalOutput").ap()

    zc_bounce = nc.dram_tensor("zc_bounce", [C.tgt_per, D], BF16).ap()
    zc_space = "Shared" if C.ncores > 4 else None
    zc_table = nc.dram_tensor("zc_table", [C.n_src, D], BF16,
                              addr_space=zc_space).ap()

    with tile.TileContext(nc) as tc, ExitStack() as top:
        const = top.enter_context(tc.tile_pool(name="const", bufs=1))

        wfc_sb = const.tile([128, KT, D], BF16)
        nc.sync.dma_start(wfc_sb[:], wfc[:, :, :])

        # ---------------- phase 0: z rows -> zc_bounce -> AllGather
        with ExitStack() as p0:
            ps0 = p0.enter_context(tc.tile_pool(name="ps0", bufs=2, space="PSUM"))
            zr_pool = p0.enter_context(tc.tile_pool(name="zrow", bufs=2))
            shp = p0.enter_context(tc.tile_pool(name="shp", bufs=1))
            sh_sb = shp.tile([128, KT, NPAD], BF16)
            nc.sync.dma_start(sh_sb[:], src_hT[:, :, :])
            for nb in range(NBLK):
                rows = min(128, C.tgt_per - nb * 128)
                if rows <= 0:
                    break
                z_ps = ps0.tile([128, D], F32, tag="zps")
                for kt in range(KT):
                    nc.tensor.matmul(z_ps[:], sh_sb[:, kt, nb * 128:(nb + 1) * 128],
                                     wfc_sb[:, kt, :], start=(kt == 0),
                                     stop=(kt == KT - 1))
                zrow = zr_pool.tile([128, D], BF16, tag="zrow")
                nc.scalar.activation(zrow[:], z_ps[:],
                                     mybir.ActivationFunctionType.Copy)
                nc.sync.dma_start(zc_bounce[nb * 128:nb * 128 + rows, :],
                                  zrow[0:rows, :])

        nc.gpsimd.collective_compute(
            "AllGather", mybir.AluOpType.bypass,
            replica_groups=[list(range(C.ncores))],
            ins=[zc_bounce[:, :]], outs=[zc_table[:, :]],
        )

        # small consts (needed right at CC end; negligible traffic)
        v_sb = const.tile([128, H], BF16)
        nc.sync.dma_start(v_sb[:], vmat[:, :])
        idx_sb = const.tile([128, TOTC * 8], I16)
        nc.sync.dma_start(idx_sb[:], idxw[:, :])
        b1_sb = const.tile([128, MT1], F32)
        nc.sync.dma_start(b1_sb[:], b1c[:, :])
        b2_sb = const.tile([128, FT], F32)
        nc.sync.dma_start(b2_sb[:], b2c[:, :])
        grep_sb = const.tile([128, D], BF16)
        nc.sync.dma_start(grep_sb[:], g_rep[:, :])
        brep_sb = const.tile([128, D], F32)
        nc.sync.dma_start(brep_sb[:], b_rep[:, :])
        idb_sb = const.tile([128, 128], BF16)
        nc.sync.dma_start(idb_sb[:], identb[:, :])
        eps_sb = const.tile([128, 1], F32)
        nc.vector.memset(eps_sb[:], LN_EPS)
        ones_sb = const.tile([128, D], BF16)
        nc.vector.memset(ones_sb[:], 1.0)

        # gate: RAW on the collective; the in-order sync queue then holds all
        # later dma_starts until the CC is done, keeping the CC exchange free
        # of heavy DMA contention
        ccgate = const.tile([128, D], BF16)
        nc.sync.dma_start(ccgate[:], zc_table[0:128, :])

        w1_sb = const.tile([128, KT, MT1, 128], BF16)
        nc.sync.dma_start(w1_sb[:], w1[:, :, :, :])
        w2_sb = const.tile([128, MT1, FT, 128], BF16)
        nc.sync.dma_start(w2_sb[:], w2[:, :, :, :])

        with ExitStack() as pb:
            keep = pb.enter_context(tc.tile_pool(name="keep", bufs=NBLK))

            # -------- score path: independent of the z table / AllGather,
            # emitted before the collective so its DMA doesn't contend
            eexps = []
            with ExitStack() as sp:
                ps_s2 = sp.enter_context(
                    tc.tile_pool(name="ps_s2", bufs=1, space="PSUM"))
                eep = sp.enter_context(tc.tile_pool(name="eep", bufs=2))
                s1pp = sp.enter_context(tc.tile_pool(name="s1p", bufs=2))
                e1p = sp.enter_context(tc.tile_pool(name="e1p", bufs=2))
                for nb in range(NBLK):
                    CPB = CPBS[nb]
                    o0 = COFF[nb]
                    ee_t = eep.tile([128, MAXC, 128], BF16, tag="ee")
                    nc.sync.dma_start(ee_t[:, 0:CPB, :], eeT[:, o0:o0 + CPB, :])
                    s1_t = s1pp.tile([128, MAXC, 8], BF16, tag="s1")
                    nc.sync.dma_start(s1_t[:, 0:CPB, :], s1b[:, o0:o0 + CPB, :])
                    s2_ps = ps_s2.tile([128, MAXC * H], F32, tag="s2")
                    for j in range(CPB):
                        nc.tensor.matmul(s2_ps[:, j * H:(j + 1) * H], ee_t[:, j, :],
                                         v_sb[:, :], start=True, stop=True)
                    e1 = e1p.tile([128, MAXC, H], F32, tag="e1")
                    nc.vector.tensor_tensor(
                        e1[:, 0:CPB, :],
                        s2_ps[:, 0:CPB * H].rearrange("p (c h) -> p c h", h=H),
                        s1_t[:, 0:CPB, :], mybir.AluOpType.add)
                    lk = e1p.tile([128, MAXC, H], F32, tag="lk")
                    nc.vector.tensor_scalar_mul(lk[:, 0:CPB, :], e1[:, 0:CPB, :],
                                                LEAK)
                    e2 = e1p.tile([128, MAXC, H], F32, tag="e2")
                    nc.vector.tensor_tensor(e2[:, 0:CPB, :], e1[:, 0:CPB, :],
                                            lk[:, 0:CPB, :], mybir.AluOpType.max)
                    eexp = keep.tile([128, MAXC, H], BF16, tag="eexp")
                    nc.scalar.activation(eexp[:, 0:CPB, :], e2[:, 0:CPB, :],
                                         mybir.ActivationFunctionType.Exp)
                    eexps.append(eexp)

            ps_hag = pb.enter_context(tc.tile_pool(name="ps_hag", bufs=2, space="PSUM"))
            ps_den = pb.enter_context(tc.tile_pool(name="ps_den", bufs=1, space="PSUM"))
            ps_tp = pb.enter_context(tc.tile_pool(name="ps_tp", bufs=1, space="PSUM"))
            ps_a1 = pb.enter_context(tc.tile_pool(name="ps_a1", bufs=2, space="PSUM"))
            ps_o2 = pb.enter_context(tc.tile_pool(name="ps_o2", bufs=2, space="PSUM"))
            ohp = pb.enter_context(tc.tile_pool(name="ohp", bufs=3))
            tgtp = pb.enter_context(tc.tile_pool(name="tgtp", bufs=2))
            zgp = pb.enter_context(tc.tile_pool(name="zg", bufs=4))
            hpool = pb.enter_context(tc.tile_pool(name="hb", bufs=2))
            hbtp = pb.enter_context(tc.tile_pool(name="hbt", bufs=1))
            fpool = pb.enter_context(tc.tile_pool(name="ffn", bufs=1))
            r1p = pb.enter_context(tc.tile_pool(name="r1", bufs=1))
            tmpp = pb.enter_context(tc.tile_pool(name="tmp", bufs=2))
            lnp = pb.enter_context(tc.tile_pool(name="ln", bufs=2))
            stp = pb.enter_context(tc.tile_pool(name="stat", bufs=2))

            hbT = hbtp.tile([128, FT, NPAD], BF16)
            r2 = fpool.tile([128, FT, NPAD], BF16)

            # -------- main block loop
            GMAX = 8  # 1024 idxs per gather; 640/768 proven on HW, 1152+ crashes
            ffn_plan = {1: [(0, 256)], 3: [(256, 256)], 5: [(512, 256)],
                        7: [(768, 256)], 9: [(1024, 256)]}

            def emit_ffn_chunk(cs, cw):
                r1 = r1p.tile([128, MT1, 512], BF16, tag="r1")
                for mt in range(MT1):
                    a1 = ps_a1.tile([128, cw], F32, tag="a1")
                    for kt in range(KT):
                        nc.tensor.matmul(a1[:], w1_sb[:, kt, mt, :],
                                         hbT[:, kt, cs:cs + cw],
                                         start=(kt == 0), stop=(kt == KT - 1))
                    nc.scalar.activation(r1[:, mt, 0:cw], a1[:],
                                         mybir.ActivationFunctionType.Relu,
                                         bias=b1_sb[:, mt:mt + 1])
                for ft in range(FT):
                    o2 = ps_o2.tile([128, cw], F32, tag="o2")
                    for kt2 in range(MT1):
                        nc.tensor.matmul(o2[:], w2_sb[:, kt2, ft, :],
                                         r1[:, kt2, 0:cw],
                                         start=(kt2 == 0), stop=(kt2 == MT1 - 1))
                    t1 = tmpp.tile([128, cw], BF16, tag="t1")
                    nc.scalar.activation(t1[:], o2[:],
                                         mybir.ActivationFunctionType.Identity,
                                         bias=b2_sb[:, ft:ft + 1])
                    nc.vector.tensor_tensor(r2[:, ft, cs:cs + cw], t1[:],
                                            hbT[:, ft, cs:cs + cw],
                                            mybir.AluOpType.add)

            def emit_ln(nb):
                tp2 = ps_tp.tile([128, D], BF16, tag="tp")
                for ft in range(FT):
                    nc.tensor.transpose(tp2[:, ft * 128:(ft + 1) * 128],
                                        r2[:, ft, nb * 128:(nb + 1) * 128],
                                        idb_sb[:])
                r2n = lnp.tile([128, D], BF16, tag="r2n")
                nc.vector.tensor_copy(r2n[:], tp2[:])
                ssum = stp.tile([128, 1], F32, tag="ssum")
                nc.vector.tensor_reduce(ssum[:], r2n[:], mybir.AxisListType.X,
                                        mybir.AluOpType.add)
                sqs = lnp.tile([128, D], BF16, tag="sqs")
                ssq = stp.tile([128, 1], F32, tag="ssq")
                nc.scalar.activation(sqs[:], r2n[:],
                                     mybir.ActivationFunctionType.Square,
                                     accum_out=ssq[:])
                mu_n = stp.tile([128, 1], F32, tag="mu_n")
                nc.vector.tensor_scalar_mul(mu_n[:], ssum[:], -1.0 / D)
                musq = stp.tile([128, 1], F32, tag="musq")
                nc.vector.tensor_tensor(musq[:], mu_n[:], mu_n[:],
                                        mybir.AluOpType.mult)
                var = stp.tile([128, 1], F32, tag="var")
                nc.vector.tensor_scalar(var[:], ssq[:], 1.0 / D, None,
                                        mybir.AluOpType.mult)
                var2 = stp.tile([128, 1], F32, tag="var2")
                nc.vector.tensor_tensor(var2[:], var[:], musq[:],
                                        mybir.AluOpType.subtract)
                std = stp.tile([128, 1], F32, tag="std")
                nc.scalar.activation(std[:], var2[:],
                                     mybir.ActivationFunctionType.Sqrt,
                                     bias=eps_sb[:, :])
                rstd = stp.tile([128, 1], F32, tag="rstd")
                nc.vector.reciprocal(rstd[:], std[:])
                nsh = stp.tile([128, 1], F32, tag="nsh")
                nc.vector.tensor_tensor(nsh[:], mu_n[:], rstd[:],
                                        mybir.AluOpType.mult)
                y = lnp.tile([128, D], BF16, tag="y")
                nc.vector.tensor_scalar(y[:], r2n[:], rstd[:], nsh[:],
                                        mybir.AluOpType.mult,
                                        mybir.AluOpType.add)
                xg = lnp.tile([128, D], BF16, tag="xg")
                nc.vector.tensor_tensor(xg[:], y[:], grep_sb[:],
                                        mybir.AluOpType.mult)
                orow = lnp.tile([128, D], F32, tag="orow")
                nc.vector.tensor_tensor(orow[:], xg[:], brep_sb[:],
                                        mybir.AluOpType.add)
                nc.sync.dma_start(out_shard[nb * 128:(nb + 1) * 128, :], orow[:])

            zgs_t = {}
            hag_t = {}
            rec_t = {}
            oh_ts = {}
            tgt_ts = {}

            for nb in range(NBLK):
                CPB = CPBS[nb]
                o0 = COFF[nb]
                zg = zgp.tile([128, MAXC, D], BF16, tag="zg")
                for g0 in range(0, CPB, GMAX):
                    gn = min(GMAX, CPB - g0)
                    nc.gpsimd.dma_gather(
                        out_ap=zg[:, g0:g0 + gn, :], in_ap=zc_table[:, :],
                        idxs_ap=idx_sb[:, (o0 + g0) * 8:(o0 + g0 + gn) * 8],
                        num_idxs=gn * 128, num_idxs_reg=gn * 128, elem_size=D)
                zgs_t[nb] = zg
                oh_t = ohp.tile([128, MAXC, 128], BF16, tag="oh")
                nc.sync.dma_start(oh_t[:, 0:CPB, :], oh[:, o0:o0 + CPB, :])
                oh_ts[nb] = oh_t
                tgtb = tgtp.tile([128, D], BF16, tag="tgtb")
                nc.sync.dma_start(tgtb[:], tgt_hm1[nb * 128:(nb + 1) * 128, :])
                tgt_ts[nb] = tgtb

            def stage_a(nb):
                CPB = CPBS[nb]
                zg = zgs_t[nb]
                oh_t = oh_ts[nb]
                eexp = eexps[nb]
                # zg *= eexp (q-order: inner dim h contiguous on both), in place
                nc.vector.tensor_tensor(
                    zg[:, 0:CPB, :].rearrange("p c (o h) -> p c o h", h=H),
                    zg[:, 0:CPB, :].rearrange("p c (o h) -> p c o h", h=H),
                    eexp[:, 0:CPB, :].rearrange("p c (h x) -> p c x h", x=1)
                        .broadcast_to([128, CPB, O, H]),
                    mybir.AluOpType.mult)
                hag = ps_hag.tile([128, D], F32, tag="hag")
                den = ps_den.tile([128, H], F32, tag="den")
                for j in range(CPB):
                    nc.tensor.matmul(hag[:], oh_t[:, j, :], zg[:, j, :],
                                     start=(j == 0), stop=(j == CPB - 1),
                                     skip_group_check=True)
                    nc.tensor.matmul(den[:], oh_t[:, j, :], eexp[:, j, :],
                                     start=(j == 0), stop=(j == CPB - 1),
                                     skip_group_check=True)
                den2 = stp.tile([128, H], F32, tag="den2")
                nc.vector.tensor_scalar_max(den2[:], den[:], 1e-30)
                rec = stp.tile([128, H], F32, tag="rec")
                nc.vector.reciprocal(rec[:], den2[:])
                hag_t[nb] = hag
                rec_t[nb] = rec

            def stage_b(nb):
                hag, rec, tgtb = hag_t[nb], rec_t[nb], tgt_ts[nb]
                hbp = hpool.tile([128, D], BF16, tag="hbp")
                nc.vector.tensor_tensor(
                    hbp[:, :].rearrange("p (h o) -> p h o", h=H),
                    hag[:, :].rearrange("p (o h) -> p h o", h=H),
                    rec[:, :].rearrange("p (h x) -> p h x", x=1)
                        .broadcast_to([128, H, O]),
                    mybir.AluOpType.mult)
                # elu(x) + tgt = relu(x) + min(exp(x),1) - 1 + tgt
                ex = hpool.tile([128, D], BF16, tag="ex")
                nc.scalar.activation(ex[:], hbp[:],
                                     mybir.ActivationFunctionType.Exp)
                u = hpool.tile([128, D], BF16, tag="u")
                nc.vector.tensor_tensor(u[:], ex[:], ones_sb[:],
                                        mybir.AluOpType.min)
                v = hpool.tile([128, D], BF16, tag="v")
                nc.scalar.activation(v[:], hbp[:],
                                     mybir.ActivationFunctionType.Relu)
                w = hpool.tile([128, D], BF16, tag="w")
                nc.vector.tensor_tensor(w[:], u[:], tgtb[:],
                                        mybir.AluOpType.add)
                hb3 = hpool.tile([128, D], BF16, tag="hb3")
                nc.vector.tensor_tensor(hb3[:], w[:], v[:],
                                        mybir.AluOpType.add)
                tp = ps_tp.tile([128, D], BF16, tag="tp")
                for ft in range(FT):
                    nc.tensor.transpose(tp[:, ft * 128:(ft + 1) * 128],
                                        hb3[:, ft * 128:(ft + 1) * 128],
                                        idb_sb[:])
                nc.vector.tensor_copy(
                    hbT[:, :, nb * 128:(nb + 1) * 128],
                    tp[:].rearrange("p (f m) -> p f m", m=128))

            stage_a(0)
            for nb in range(1, NBLK):
                stage_a(nb)
                stage_b(nb - 1)
                for cs, cw in ffn_plan.get(nb - 1, []):
                    emit_ffn_chunk(cs, cw)
                    for nb_ln in range(cs // 128, (cs + cw) // 128):
                        emit_ln(nb_ln)
            stage_b(NBLK - 1)
            for cs, cw in ffn_plan.get(NBLK - 1, []):
                emit_ffn_chunk(cs, cw)
                for nb_ln in range(cs // 128, (cs + cw) // 128):
                    emit_ln(nb_ln)

    nc.compile()
    return nc


_CACHE = {}


def _get_program(C):
    key = (C.ncores, C.n_src, C.n_tgt, C.e, C.cpbs)
    if key not in _CACHE:
        _CACHE[key] = build_program(C)
    return _CACHE[key]


def kernel(src_h, tgt_h, edge_embed, edge_src, edge_dst,
           W_fc, W_feat, attn_a, w1, b1, w2, b2, ln_g, ln_b):
    from concourse.bass_utils import run_bass_kernel_spmd

    C = full_cfg()
    cores, shared = host_prep(C, src_h, tgt_h, edge_embed, edge_src, edge_dst,
                              W_fc, W_feat, attn_a, w1, b1, w2, b2, ln_g, ln_b)
    nc = _get_program(C)
    in_maps = []
    for c in range(C.ncores):
        m = dict(shared)
        cc = cores[c]
        m.update(idxw=cc["idxw"], eeT=cc["eeT"], oh=cc["oh"], s1b=cc["s1b"],
                 tgt_hm1=cc["tgt_hm1"], src_hT=cc["src_hT"])
        in_maps.append(m)
    import os
    try:
        res = run_bass_kernel_spmd(nc, in_maps, list(range(C.ncores)))
    except Exception:
        if os.environ.get("BASS_TRACE"):
            os.environ["BASS_NEVER_TRACE"] = "1"
            res = run_bass_kernel_spmd(nc, in_maps, list(range(C.ncores)))
        else:
            raise
    global _last_results
    _last_results = res
    out = np.zeros((C.n_tgt, C.d), np.float32)
    for c in range(C.ncores):
        shard = res.results[c]["out_shard"]
        for s, (lo, hi) in enumerate(cores[c]["bounds"]):
            if hi > lo:
                out[lo:hi] = shard[s * 128:s * 128 + (hi - lo)]
    return np.ascontiguousarray(out, dtype=np.float32)
